# revision 31
# baseline (speedup 1.0000x reference)
"""Trainium2 Bass kernel for nn_AttentionModule (dense transformer block).

Computation (per batch element b):
    q = X @ Wq.T ; k = K @ Wk.T ; v = X @ Wv.T        (X=query_input, K=key_input)
    a = softmax((k @ q.T) / sqrt(D), axis=-1)          -> (NK, NQ)
    out = a @ v + K                                    -> (NK, D)

Sharding: data-parallel over batch, one batch element per NeuronCore (B == 8).

All five matmul stages run as fp8e4m3 DoubleRow matmuls (two 128-deep
k-tiles per pass, ~2x PE throughput vs bf16); accumulation stays fp32 in
PSUM, exp runs on the scalar engine from the fp32 scores, and the softmax
normalization (per-partition reciprocal scale on ACT) + fp32 residual add
of key_input (DVE) run in the output pass, so overall error stays at the
~1e-3 level (tolerance 2e-2). With fp8 operands, qT, kT and v (2 MB each)
stay SBUF-resident — no DRAM spills. Operands are laid out as [128, 2, n]
"pair" tiles: dim1 selects which of the two k-tiles a DoubleRow matmul
consumes; input pair tiles fill with one rearranged DMA each, spread
across the SP/ACT/Pool DGE queues in first-consumed order. Matmul groups
pair two 512-wide outputs into one [P, 1024] two-bank PSUM tile so each
PSUM evacuation (alternating DVE/ACT) and each exp covers 1024 elements,
halving the elementwise-op and semaphore count. The column sums for the
softmax denominator ride a DoubleRow ones-vector matmul (ones padded to
16 columns for the dual-fp8 LDWEIGHTS stride rule), emitted one pair late
so the exp->colsum dependency never stalls the PE.

Cost-model (TimelineSim) estimate: ~126 us/core vs ~412 us for the bf16
baseline; rel err vs the fp32 reference 8.9e-4.
"""

import numpy as np
import ml_dtypes

import concourse.tile as tile
from concourse import bacc, mybir
from concourse.bass_utils import run_bass_kernel_spmd
from concourse.masks import make_identity

B, NQ, NK, D = 8, 2048, 2048, 1024
P = 128
EB = D // P          # 8 feature blocks
DP = EB // 2         # 4 feature-block pairs (DoubleRow granularity)
NB = NQ // P         # 16 query-row blocks
NP = NB // 2         # 8 query-row pairs
MC = 512             # scores chunk width (n_k columns per chunk)
NMC = NK // MC       # 4 chunks
SCALE = 1.0 / float(np.sqrt(np.float32(D)))

F32 = mybir.dt.float32
FP8 = mybir.dt.float8e4
DR = mybir.MatmulPerfMode.DoubleRow

_CACHE = {}


def _build():
    nc = bacc.Bacc("TRN2", target_bir_lowering=False, debug=False, num_devices=B)

    x8 = nc.dram_tensor("x8", [D, NQ], FP8, kind="ExternalInput").ap()
    kt8 = nc.dram_tensor("kt8", [D, NK], FP8, kind="ExternalInput").ap()
    knat = nc.dram_tensor("knat", [NK, D], F32, kind="ExternalInput").ap()
    wq8 = nc.dram_tensor("wq8", [D, D], FP8, kind="ExternalInput").ap()
    wk8 = nc.dram_tensor("wk8", [D, D], FP8, kind="ExternalInput").ap()
    wv8 = nc.dram_tensor("wv8", [D, D], FP8, kind="ExternalInput").ap()
    out = nc.dram_tensor("out", [NK, D], F32, kind="ExternalOutput").ap()

    with tile.TileContext(nc) as tc:
        with (
            tc.tile_pool(name="const", bufs=1) as constp,
            tc.tile_pool(name="qt", bufs=DP) as qtp,
            tc.tile_pool(name="kt", bufs=DP) as ktp,
            tc.tile_pool(name="vt", bufs=NP) as vtp,
        ):
            ident = constp.tile([1, 1], F32, tag="ident", name="ident")
            make_identity(nc, ident)
            # ones padded to 16 columns: the DoubleRow LDWEIGHTS ISA check
            # requires the pair-dim stride to be a multiple of 16 bytes
            ones_f = constp.tile([P, 2, 16], F32, tag="onesf", name="onesf")
            nc.vector.memset(ones_f, 1.0)
            ones8 = constp.tile([P, 2, 16], FP8, tag="ones", name="ones")
            nc.vector.tensor_copy(ones8, ones_f)

            # persistent fp8 pair tiles: dim1 = which k-tile of a DoubleRow pair
            qT = [qtp.tile([P, 2, NQ], FP8, tag="qt", name="qt") for _ in range(DP)]
            kT = [ktp.tile([P, 2, NK], FP8, tag="kt", name="kt") for _ in range(DP)]
            vS = [vtp.tile([P, 2, D], FP8, tag="vt", name="vt") for _ in range(NP)]

            # ---------------- phase 1: projections ----------------
            with (
                tc.tile_pool(name="bigin", bufs=2 * DP) as bigp,
                tc.tile_pool(name="wpool", bufs=3 * DP) as wp,
                tc.tile_pool(name="psum1", bufs=2, space="PSUM") as psp,
            ):
                # loads, in first-consumed order: wk, ktT halves, x, wq, wv.
                # each pair tile fills with ONE rearranged DMA (row-block pair
                # interleave) to keep the HWDGE queue short.
                # the three HWDGE/SWDGE queues issue in parallel so the first
                # k-proj group's operands (all of wk + kt first half) land fast
                def pair_load(pool, src, dp, tag, eng, cols=None, bufs=None):
                    t = pool.tile([P, 2, src.shape[1]], FP8, tag=tag, name=tag,
                                  bufs=bufs)
                    s = src[2 * dp * P:(2 * dp + 2) * P, :]
                    if cols is None:
                        eng.dma_start(
                            out=t, in_=s.rearrange("(two p) d -> p two d", two=2)
                        )
                    else:
                        eng.dma_start(
                            out=t[:, :, cols[0]:cols[1]],
                            in_=s[:, cols[0]:cols[1]].rearrange(
                                "(two p) d -> p two d", two=2),
                        )
                    return t

                wk_pr = [pair_load(wp, wk8, dp, "w", nc.sync, bufs=3 * DP)
                         for dp in range(DP)]
                kt_pr = [pair_load(bigp, kt8, dp, "big", nc.scalar,
                                   cols=(0, NK // 2), bufs=2 * DP)
                         for dp in range(DP)]
                for dp in range(DP):
                    nc.gpsimd.dma_start(
                        out=kt_pr[dp][:, :, NK // 2:NK],
                        in_=kt8[2 * dp * P:(2 * dp + 2) * P, NK // 2:NK].rearrange(
                            "(two p) d -> p two d", two=2),
                    )
                x_pr = [pair_load(bigp, x8, dp, "big", nc.gpsimd, bufs=2 * DP)
                        for dp in range(DP)]
                wq_pr = [pair_load(wp, wq8, dp, "w", nc.sync, bufs=3 * DP)
                         for dp in range(DP)]
                wv_pr = [pair_load(wp, wv8, dp, "w", nc.sync, bufs=3 * DP)
                         for dp in range(DP)]

                # each projection pairs two 512-wide matmul groups into one
                # [P, 1024] two-bank PSUM tile and evacuates both with ONE
                # strided copy into the destination pair tile; the copies
                # alternate DVE / ACT so neither engine gates the PE
                def evac(dst, ps, gi):
                    if gi % 2 == 0:
                        nc.vector.tensor_copy(dst, ps)
                    else:
                        nc.scalar.activation(
                            out=dst, in_=ps,
                            func=mybir.ActivationFunctionType.Copy,
                        )

                # -- kT[e, m] = sum_d wk[d, e] * ktT[d, m]  (SBUF resident)
                gi = 0
                for mc4 in range(NK // 512):
                    for ep in range(DP):
                        tg = "mm" if gi % 2 == 0 else "st"
                        ps = psp.tile([P, 1024], F32, tag=tg, name="mm", bufs=2)
                        for half in range(2):
                            eb = 2 * ep + half
                            for dp in range(DP):
                                nc.tensor.matmul(
                                    ps[:, half * 512:(half + 1) * 512],
                                    wk_pr[dp][:, :, eb * P:(eb + 1) * P],
                                    kt_pr[dp][:, :, mc4 * 512:(mc4 + 1) * 512],
                                    start=(dp == 0),
                                    stop=(dp == DP - 1),
                                    perf_mode=DR,
                                )
                        evac(kT[ep][:, :, mc4 * 512:(mc4 + 1) * 512],
                             ps.rearrange("p (two d) -> p two d", two=2), gi)
                        gi += 1

                # -- qT[e, n] = sum_d wq[d, e] * xT[d, n]  (SBUF resident)
                for nc4 in range(NQ // 512):
                    for ep in range(DP):
                        tg = "mm" if gi % 2 == 0 else "st"
                        ps = psp.tile([P, 1024], F32, tag=tg, name="mm", bufs=2)
                        for half in range(2):
                            eb = 2 * ep + half
                            for dp in range(DP):
                                nc.tensor.matmul(
                                    ps[:, half * 512:(half + 1) * 512],
                                    wq_pr[dp][:, :, eb * P:(eb + 1) * P],
                                    x_pr[dp][:, :, nc4 * 512:(nc4 + 1) * 512],
                                    start=(dp == 0),
                                    stop=(dp == DP - 1),
                                    perf_mode=DR,
                                )
                        evac(qT[ep][:, :, nc4 * 512:(nc4 + 1) * 512],
                             ps.rearrange("p (two d) -> p two d", two=2), gi)
                        gi += 1

                # -- v[n, dv] = sum_d xT[d, n] * wv[d, dv]  (SBUF resident)
                for np_ in range(NP):
                    for dc in range(D // 512):
                        tg = "mm" if gi % 2 == 0 else "st"
                        ps = psp.tile([P, 1024], F32, tag=tg, name="mm", bufs=2)
                        for half in range(2):
                            nb = 2 * np_ + half
                            for dp in range(DP):
                                nc.tensor.matmul(
                                    ps[:, half * 512:(half + 1) * 512],
                                    x_pr[dp][:, :, nb * P:(nb + 1) * P],
                                    wv_pr[dp][:, :, dc * 512:(dc + 1) * 512],
                                    start=(dp == 0),
                                    stop=(dp == DP - 1),
                                    perf_mode=DR,
                                )
                        evac(vS[np_][:, :, dc * 512:(dc + 1) * 512],
                             ps.rearrange("p (two d) -> p two d", two=2), gi)
                        gi += 1

            # ---------------- phase 2: attention ----------------
            # chunks are software-pipelined one deep: scores(c+1) is emitted
            # before context(c), so each chunk's colsum -> reciprocal ->
            # transpose chain overlaps the next chunk's score matmuls and the
            # ACT-heavy score pass interleaves with the PE-heavy context pass.
            with (
                tc.tile_pool(name="expst", bufs=20) as expp,
                tc.tile_pool(name="knp", bufs=3) as knp,
                tc.tile_pool(name="outp", bufs=3) as outp,
                tc.tile_pool(name="ctmp", bufs=4) as ctmpp,
                tc.tile_pool(name="small", bufs=4) as smallp,
                tc.tile_pool(name="psum2", bufs=2, space="PSUM") as psp,
            ):
                def scores_chunk(mc):
                    # two row-blocks share one [P, 1024] two-bank PSUM tile;
                    # ONE exp activation then fills a whole expst pair tile
                    m0 = mc * MC
                    expst = [expp.tile([P, 2, MC], FP8, tag="expst", name="expst")
                             for _ in range(NP)]
                    cs_ps = psp.tile([1, MC], F32, tag="csrp", name="cs", bufs=1)
                    for jp in range(NP):
                        st_ps = psp.tile([P, 1024], F32, tag="st", name="st", bufs=2)
                        for half in range(2):
                            nb = 2 * jp + half
                            for ep in range(DP):
                                nc.tensor.matmul(
                                    st_ps[:, half * 512:(half + 1) * 512],
                                    qT[ep][:, :, nb * P:(nb + 1) * P],
                                    kT[ep][:, :, m0:m0 + MC],
                                    start=(ep == 0),
                                    stop=(ep == DP - 1),
                                    perf_mode=DR,
                                )
                        nc.scalar.activation(
                            out=expst[jp],
                            in_=st_ps.rearrange("p (two d) -> p two d", two=2),
                            func=mybir.ActivationFunctionType.Exp, scale=SCALE,
                        )
                        # the column-sum matmul for pair j is emitted one pair
                        # late so the exp -> cs semaphore never gates PE
                        if jp >= 1:
                            j = jp - 1
                            nc.tensor.matmul(
                                cs_ps, ones8[:, :, 0:1], expst[j],
                                start=(j == 0), stop=False, perf_mode=DR,
                            )
                    nc.tensor.matmul(
                        cs_ps, ones8[:, :, 0:1], expst[NP - 1],
                        start=False, stop=True, perf_mode=DR,
                    )
                    recip_row = smallp.tile([1, MC], F32, tag="rrow", name="rrow")
                    nc.vector.reciprocal(recip_row, cs_ps)
                    rp_ps = psp.tile([P, MC // P], F32, tag="csrp", name="rp", bufs=1)
                    for j in range(MC // P):
                        nc.tensor.transpose(
                            rp_ps[:, j:j + 1],
                            recip_row[:, j * P:(j + 1) * P],
                            ident,
                        )
                    recip_pp = smallp.tile([P, MC // P], F32, tag="rpp", name="rpp")
                    nc.vector.tensor_copy(recip_pp, rp_ps)
                    return expst, recip_pp

                def context_chunk(mc, expst, recip_pp):
                    # context: C[m, dv] = sum_n expst[n, m] * v[n, dv]
                    # normalization splits into an ACT scaled-copy (per-
                    # partition recip) + a DVE residual add; two row-blocks
                    # share one knat load / one out store (SWDGE carries knat
                    # so the HWDGE queues stay short).
                    m0 = mc * MC
                    for mp in range(MC // (2 * P)):
                        r0 = m0 + mp * 2 * P
                        kn = knp.tile([P, 2, D], F32, tag="knat", name="knat")
                        nc.gpsimd.dma_start(
                            out=kn,
                            in_=knat[r0:r0 + 2 * P, :].rearrange(
                                "(two p) d -> p two d", two=2),
                        )
                        ot = outp.tile([P, 2, D], F32, tag="ostage", name="ostage")
                        for half in range(2):
                            msb = 2 * mp + half
                            cn = ctmpp.tile([P, D], F32, tag="ctmp", name="ctmp")
                            for dc in range(D // 512):
                                c_ps = psp.tile([P, 512], F32, tag="mm",
                                                name="mm", bufs=3)
                                for np_ in range(NP):
                                    nc.tensor.matmul(
                                        c_ps,
                                        expst[np_][:, :, msb * P:(msb + 1) * P],
                                        vS[np_][:, :, dc * 512:(dc + 1) * 512],
                                        start=(np_ == 0),
                                        stop=(np_ == NP - 1),
                                        perf_mode=DR,
                                    )
                                nc.scalar.activation(
                                    out=cn[:, dc * 512:(dc + 1) * 512], in_=c_ps,
                                    func=mybir.ActivationFunctionType.Copy,
                                    scale=recip_pp[:, msb:msb + 1],
                                )
                            nc.vector.tensor_tensor(
                                out=ot[:, half, :],
                                in0=cn,
                                in1=kn[:, half, :],
                                op=mybir.AluOpType.add,
                            )
                        if mc == NMC - 1 and mp == MC // (2 * P) - 1:
                            # last block: two half stores so the final DMA is
                            # small and the kernel tail short
                            nc.scalar.dma_start(
                                out=out[r0:r0 + P, :], in_=ot[:, 0, :])
                            nc.scalar.dma_start(
                                out=out[r0 + P:r0 + 2 * P, :], in_=ot[:, 1, :])
                        else:
                            nc.scalar.dma_start(
                                out=out[r0:r0 + 2 * P, :].rearrange(
                                    "(two p) d -> p two d", two=2),
                                in_=ot,
                            )

                for mc in range(NMC):
                    expst, recip_pp = scores_chunk(mc)
                    context_chunk(mc, expst, recip_pp)

    nc.compile()
    return nc


def _get_nc():
    if "nc" not in _CACHE:
        _CACHE["nc"] = _build()
    return _CACHE["nc"]


def _in_maps(query_input, key_input, Wq, Wk, Wv):
    f8 = ml_dtypes.float8_e4m3
    query_input = np.asarray(query_input, dtype=np.float32)
    key_input = np.asarray(key_input, dtype=np.float32)
    wq8 = np.ascontiguousarray(np.asarray(Wq, dtype=np.float32).T).astype(f8)
    wk8 = np.ascontiguousarray(np.asarray(Wk, dtype=np.float32).T).astype(f8)
    wv8 = np.ascontiguousarray(np.asarray(Wv, dtype=np.float32).T).astype(f8)
    in_maps = []
    for b in range(B):
        in_maps.append({
            "x8": np.ascontiguousarray(query_input[b].T).astype(f8),
            "kt8": np.ascontiguousarray(key_input[b].T).astype(f8),
            "knat": np.ascontiguousarray(key_input[b]),
            "wq8": wq8,
            "wk8": wk8,
            "wv8": wv8,
        })
    return in_maps


def kernel(query_input, key_input, Wq, Wk, Wv):
    nc = _get_nc()
    in_maps = _in_maps(query_input, key_input, Wq, Wk, Wv)
    res = run_bass_kernel_spmd(nc, in_maps, list(range(B))).results
    return np.stack([res[b]["out"] for b in range(B)], axis=0)
